# revision 3
# baseline (speedup 1.0000x reference)
"""Multi-head cross-attention (MHAForCrossFusion) on 8 Trainium2 cores.

Strategy: tensor-parallel over heads. Each core owns 2 of the 16 heads:
 - column slices of Wq/Wk/Wv (128 features each), row slice of Wo
 - q/k/v replicated; each core produces a full-shape partial of the
   output projection; host gathers by summing the 8 partials (+ bo).

Per-core device program:
 - stream q/k/v row-tiles in, PE-transpose to feature-major
 - projections: qm/km feature-major [128h, 4096t]; vm token-major
   [t, hv] with an appended ones column (softmax denominator trick)
 - scores S.T = km.T @ qm per head (K=64, two heads row-packed in the
   PE array), exp via ACT with the 1/sqrt(hd) scale folded in
 - ctx_aug[0:65] = [vm | 1].T @ expS accumulated over key tiles:
   rows 0:64 = unnormalized context, row 64 = softmax denominator
 - normalize: DVE reciprocal of the denom row, K=1 matmul broadcast
   across partitions, DVE multiply
 - out-projection: out[t, :] = ctx_norm.T @ Wo_slice.T  (partial sum)
"""

import os

import numpy as np

import concourse.bass as bass
import concourse.mybir as mybir
import concourse.tile as tile
from concourse import bass_utils
from concourse.masks import make_identity

N_CORES = 8
B, L, D = 2, 2048, 1024
T = B * L  # 4096 flattened tokens; batches are disjoint 2048-token ranges
NH, HD = 16, 64
CW = (NH // N_CORES) * HD  # 128 features per core (2 heads)
SCALE = 1.0 / np.sqrt(HD)

# matmul compute dtype: float32r is ~4x faster on the PE at N>=256
USE_F32R = False

F32 = mybir.dt.float32


def _r(ap):
    return ap.bitcast(mybir.dt.float32r) if USE_F32R else ap


def _split_matmul_waits(nc):
    """fp32/fp32r matmuls lower to a self-loading LDW whose ISA struct has a
    single sem-wait slot (HWDGE DMA likewise); walrus rejects >1 wait. Move
    extra waits onto same-engine NoOps inserted right before the matmul
    (program order on the sequencer preserves the happens-before)."""
    for f in nc.m.functions:
        for bb in f.blocks:
            insts = list(bb.instructions)
            out = []
            for inst in insts:
                si = inst.sync_info
                if si is not None and len(si.on_wait) > 1:
                    for w in si.on_wait[:-1]:
                        nop = mybir.InstNoOp(
                            name=nc.get_next_instruction_name(),
                            ins=[],
                            outs=[],
                            engine=inst.engine,
                            bass_nofuse=True,
                        )
                        nop.sync_info = mybir.SyncInfo(on_wait=[w], on_update=[])
                        out.append(nop)
                    inst.sync_info = mybir.SyncInfo(
                        on_wait=[si.on_wait[-1]], on_update=si.on_update
                    )
                out.append(inst)
            if len(out) != len(insts):
                bb.instructions = out
    return nc


def build_nc():
    nc = bass.Bass("TRN2", target_bir_lowering=False, debug=False)

    qf = nc.dram_tensor("qf", [T, D], F32, kind="ExternalInput").ap()
    kf = nc.dram_tensor("kf", [T, D], F32, kind="ExternalInput").ap()
    vf = nc.dram_tensor("vf", [T, D], F32, kind="ExternalInput").ap()
    wqt = nc.dram_tensor("wqt", [D, CW], F32, kind="ExternalInput").ap()
    wkt = nc.dram_tensor("wkt", [D, CW], F32, kind="ExternalInput").ap()
    wvt = nc.dram_tensor("wvt", [D, CW], F32, kind="ExternalInput").ap()
    wot = nc.dram_tensor("wot", [CW, D], F32, kind="ExternalInput").ap()
    bq = nc.dram_tensor("bq", [CW, 1], F32, kind="ExternalInput").ap()
    bk = nc.dram_tensor("bk", [CW, 1], F32, kind="ExternalInput").ap()
    bv = nc.dram_tensor("bv", [CW, 1], F32, kind="ExternalInput").ap()
    out_p = nc.dram_tensor("out_p", [T, D], F32, kind="ExternalOutput").ap()

    DC = D // 128  # 8 contraction tiles for the projections
    NT = T // 128  # 32 token tiles
    with tile.TileContext(nc) as tc:
        with (
            tc.tile_pool(name="singles", bufs=1) as singles,
            tc.tile_pool(name="acts", bufs=1) as acts,
            tc.tile_pool(name="rows", bufs=3) as rows,
            tc.tile_pool(name="stage", bufs=2) as stage,
            tc.tile_pool(name="small", bufs=4) as small,
            tc.tile_pool(name="psum", bufs=8, space="PSUM") as pp,
        ):
            ident = singles.tile([128, 128], F32)
            make_identity(nc, ident)
            ones = singles.tile([1, 64], F32)
            nc.vector.memset(ones, 1.0)

            w_sb = {}
            for name, dram in (("wq", wqt), ("wk", wkt), ("wv", wvt)):
                w = singles.tile([128, DC, CW], F32, name=name + "_sb")
                nc.sync.dma_start(w, dram.rearrange("(c p) h -> p c h", p=128))
                w_sb[name] = w
            wot_sb = singles.tile([CW, D], F32)
            nc.sync.dma_start(wot_sb, wot)
            b_sb = {}
            for name, dram in (("bq", bq), ("bk", bk), ("bv", bv)):
                b = singles.tile([CW, 1], F32, name=name + "_sb")
                nc.sync.dma_start(b, dram)
                b_sb[name] = b

            qm = acts.tile([CW, T], F32)   # feature-major projections
            km = acts.tile([CW, T], F32)
            vma = acts.tile([128, NT, 132], F32)  # [t%128, t//128, (hv|one)x2 heads]
            ctxn = acts.tile([CW, T], F32)

            # ones columns of the augmented V (col 64 for h0, col 129 for h1)
            nc.vector.memset(
                vma.rearrange("p t (g c) -> p t g c", c=66)[:, :, :, 64], 1.0
            )

            # ---- phase 1: transpose + projections, per 256-token chunk ----
            for ci in range(T // 256):
                xT = {}
                for name, dram in (("q", qf), ("k", kf), ("v", vf)):
                    xT[name] = stage.tile(
                        [128, DC, 256], F32, tag=f"{name}T", name=f"{name}T"
                    )
                    for tt in range(2):
                        t0 = ci * 256 + tt * 128
                        row = rows.tile([128, D], F32, tag="row")
                        nc.sync.dma_start(row, dram[t0 : t0 + 128, :])
                        for dc in range(DC):
                            tp = pp.tile([128, 128], F32, tag="b")
                            nc.tensor.transpose(
                                tp, row[:, dc * 128 : (dc + 1) * 128], ident
                            )
                            nc.vector.tensor_copy(
                                xT[name][:, dc, tt * 128 : (tt + 1) * 128], tp
                            )

                for name, src, dst in (("wq", "q", qm), ("wk", "k", km)):
                    ps = pp.tile([128, 256], F32, tag="b")
                    for dc in range(DC):
                        nc.tensor.matmul(
                            ps,
                            lhsT=_r(w_sb[name][:, dc, :]),
                            rhs=_r(xT[src][:, dc, :]),
                            start=(dc == 0),
                            stop=(dc == DC - 1),
                        )
                    nc.scalar.activation(
                        dst[:, ci * 256 : (ci + 1) * 256],
                        ps,
                        mybir.ActivationFunctionType.Identity,
                        bias=b_sb["b" + name[1]],
                    )

                # V: feature-major matmul, add bias, then transpose to token-major
                ps = pp.tile([128, 256], F32, tag="b")
                for dc in range(DC):
                    nc.tensor.matmul(
                        ps,
                        lhsT=_r(w_sb["wv"][:, dc, :]),
                        rhs=_r(xT["v"][:, dc, :]),
                        start=(dc == 0),
                        stop=(dc == DC - 1),
                    )
                vmF = stage.tile([128, 256], F32, tag="vmF")
                nc.scalar.activation(
                    vmF, ps, mybir.ActivationFunctionType.Identity, bias=b_sb["bv"]
                )
                for tt in range(2):
                    tp = pp.tile([128, 128], F32, tag="b")
                    nc.tensor.transpose(tp, vmF[:, tt * 128 : (tt + 1) * 128], ident)
                    nc.vector.tensor_copy(
                        vma.rearrange("p t (g c) -> p t g c", c=66)[
                            :, ci * 2 + tt, :, 0:64
                        ],
                        tp.rearrange("p (g c) -> p g c", c=64),
                    )

            # ---- phase 2: attention + out-projection, per 512-query chunk ----
            for b in range(B):
                for lc in range(L // 512):
                    ls = slice(b * L + lc * 512, b * L + (lc + 1) * 512)
                    ctx = [
                        pp.tile([128, 512], F32, tag="b", name=f"ctx{h}")
                        for h in range(2)
                    ]
                    for pt in range(L // 128):
                        ptg = b * (L // 128) + pt
                        ps_ = slice(b * L + pt * 128, b * L + (pt + 1) * 128)
                        es = []
                        for h in range(2):
                            hs = slice(h * 64, (h + 1) * 64)
                            s = pp.tile([128, 512], F32, tag="b")
                            nc.tensor.matmul(
                                s,
                                lhsT=_r(km[hs, ps_]),
                                rhs=_r(qm[hs, ls]),
                                tile_position=(h * 64, 0),
                            )
                            e = small.tile([128, 512], F32, tag="e")
                            nc.scalar.activation(
                                e, s, mybir.ActivationFunctionType.Exp, scale=SCALE
                            )
                            es.append(e)
                        for h in range(2):
                            nc.tensor.matmul(
                                ctx[h][0:65, :],
                                lhsT=_r(vma[:, ptg, h * 66 : h * 66 + 65]),
                                rhs=_r(es[h]),
                                start=(pt == 0),
                                stop=(pt == L // 128 - 1),
                            )
                    for h in range(2):
                        rc = small.tile([1, 512], F32, tag="rc")
                        nc.vector.reciprocal(rc, ctx[h][64:65, :])
                        nc.tensor.matmul(
                            ctx[h][64:128, :], lhsT=_r(ones), rhs=_r(rc)
                        )
                        bcs = small.tile([64, 512], F32, tag="bcs")
                        nc.vector.tensor_copy(bcs, ctx[h][64:128, :])
                        nc.vector.tensor_mul(
                            ctxn[h * 64 : (h + 1) * 64, ls], ctx[h][0:64, :], bcs
                        )
                    for tt in range(4):
                        t0 = b * L + lc * 512 + tt * 128
                        ob = small.tile([128, D], F32, tag="ob")
                        for eh in range(2):
                            po = pp.tile([128, 512], F32, tag="b")
                            nc.tensor.matmul(
                                po,
                                lhsT=_r(ctxn[:, t0 : t0 + 128]),
                                rhs=_r(wot_sb[:, eh * 512 : (eh + 1) * 512]),
                            )
                            if eh == 0:
                                nc.vector.tensor_copy(ob[:, 0:512], po)
                            else:
                                nc.scalar.copy(ob[:, 512:1024], po)
                        nc.sync.dma_start(out_p[t0 : t0 + 128, :], ob)
    return _split_matmul_waits(nc)


_NC_CACHE = None


def kernel(q, k, v, attention_mask, Wq, bq, Wk, bk, Wv, bv, Wo, bo):
    global _NC_CACHE
    q, k, v = (np.asarray(x, np.float32) for x in (q, k, v))
    assert np.asarray(attention_mask).all(), "kernel assumes all-ones mask"
    if _NC_CACHE is None:
        _NC_CACHE = build_nc()
    nc = _NC_CACHE

    c = np.ascontiguousarray
    in_maps = []
    for ci in range(N_CORES):
        hs = slice(ci * CW, (ci + 1) * CW)
        in_maps.append(
            {
                "qf": q.reshape(T, D),
                "kf": k.reshape(T, D),
                "vf": v.reshape(T, D),
                "wqt": c(np.asarray(Wq, np.float32).T[:, hs]),
                "wkt": c(np.asarray(Wk, np.float32).T[:, hs]),
                "wvt": c(np.asarray(Wv, np.float32).T[:, hs]),
                "wot": c(np.asarray(Wo, np.float32).T[hs, :]),
                "bq": c(np.asarray(bq, np.float32)[hs, None]),
                "bk": c(np.asarray(bk, np.float32)[hs, None]),
                "bv": c(np.asarray(bv, np.float32)[hs, None]),
            }
        )

    res = bass_utils.run_bass_kernel_spmd(
        nc,
        in_maps,
        core_ids=list(range(N_CORES)),
        tmpdir=os.environ.get("KERNEL_TMPDIR"),
    )
    globals()["LAST_RES"] = res
    out = np.zeros((T, D), np.float32)
    for r in res.results:
        out += r["out_p"]
    out += np.asarray(bo, np.float32)[None, :]
    return out.reshape(B, L, D)



# revision 9
# speedup vs baseline: 3.7276x; 3.7276x over previous
"""Multi-head cross-attention (MHAForCrossFusion) on 8 Trainium2 cores.

Sharding: core = (batch, head-group). Core 4*b+j owns batch b and heads
4j..4j+3 (CW=256 projection features). Each core reads only its batch's
q/k/v (host pre-transposed to feature-major, cast to bf16) and writes a
full-width fp32 partial of its batch's output rows; host sums 4 partials
per batch + bo.

Per-core device program (matmul operands bf16, fp32 PSUM accumulate):
 - K/Q projections feature-major [feat, tok]; V projected token-major
   (activation tile as the stationary operand) straight into the
   ones-augmented vma layout [key, head*(hv|1)] (softmax denom trick)
 - attention per (head-pair g, 512-query chunk): scores S.T = km.T @ qm
   per head over each 128-key tile; the two heads of the pair are
   row-packed in the PE array (tile_position) and land in adjacent PSUM
   banks, so exp runs as one ACT instruction over [128, 2*512]
 - ctx_aug[0:65] = [vm | 1].T @ expS accumulated over key tiles;
   row 64 = softmax denominator
 - normalize: reciprocal of the denom row, K=1 matmul broadcast across
   partitions, DVE multiply -> ctxn
 - out-projection: out[t, :] = ctxn.T @ Wo_slice.T (partial sum)
"""

import os

import numpy as np
import ml_dtypes

import concourse.bass as bass
import concourse.mybir as mybir
import concourse.tile as tile
from concourse import bass_utils

N_CORES = 8
B, L, D = 2, 2048, 1024
NH, HD = 16, 64
HG = NH // (N_CORES // B)  # 4 heads per core
CW = HG * HD  # 256 projection features per core
SCALE = 1.0 / np.sqrt(HD)

F32 = mybir.dt.float32
BF16 = mybir.dt.bfloat16

DC = D // 128  # 8 contraction tiles for the projections
NT = L // 128  # 16 key tiles
NCH = L // 512  # 4 token chunks


def _split_matmul_waits(nc):
    """fp32/fp32r matmuls lower to a self-loading LDW whose ISA struct has a
    single sem-wait slot (HWDGE DMA likewise); walrus rejects >1 wait. Move
    extra waits onto same-engine NoOps inserted right before the matmul
    (program order on the sequencer preserves the happens-before)."""
    for f in nc.m.functions:
        for bb in f.blocks:
            insts = list(bb.instructions)
            out = []
            for inst in insts:
                si = inst.sync_info
                if si is not None and len(si.on_wait) > 1:
                    for w in si.on_wait[:-1]:
                        nop = mybir.InstNoOp(
                            name=nc.get_next_instruction_name(),
                            ins=[],
                            outs=[],
                            engine=inst.engine,
                            bass_nofuse=True,
                        )
                        nop.sync_info = mybir.SyncInfo(on_wait=[w], on_update=[])
                        out.append(nop)
                    inst.sync_info = mybir.SyncInfo(
                        on_wait=[si.on_wait[-1]], on_update=si.on_update
                    )
                out.append(inst)
            if len(out) != len(insts):
                bb.instructions = out
    return nc


def build_nc():
    nc = bass.Bass("TRN2", target_bir_lowering=False, debug=False)

    qT = nc.dram_tensor("qT", [D, L], BF16, kind="ExternalInput").ap()
    kT = nc.dram_tensor("kT", [D, L], BF16, kind="ExternalInput").ap()
    vT = nc.dram_tensor("vT", [D, L], BF16, kind="ExternalInput").ap()
    wqt = nc.dram_tensor("wqt", [D, CW], BF16, kind="ExternalInput").ap()
    wkt = nc.dram_tensor("wkt", [D, CW], BF16, kind="ExternalInput").ap()
    wvt = nc.dram_tensor("wvt", [D, CW], BF16, kind="ExternalInput").ap()
    wot = nc.dram_tensor("wot", [CW, D], BF16, kind="ExternalInput").ap()
    bq = nc.dram_tensor("bq", [CW, 1], F32, kind="ExternalInput").ap()
    bk = nc.dram_tensor("bk", [CW, 1], F32, kind="ExternalInput").ap()
    bvb = nc.dram_tensor("bvb", [128, CW], F32, kind="ExternalInput").ap()
    out_p = nc.dram_tensor("out_p", [L, D], F32, kind="ExternalOutput").ap()

    with tile.TileContext(nc) as tc:
        with (
            tc.tile_pool(name="singles", bufs=1) as singles,
            tc.tile_pool(name="acts", bufs=1) as acts,
            tc.tile_pool(name="stage", bufs=3) as stage,
            tc.tile_pool(name="small", bufs=3) as small,
            tc.tile_pool(name="psq", bufs=2, space="PSUM") as ppq,
            tc.tile_pool(name="psc", bufs=4, space="PSUM") as ppc,
        ):
            ones = singles.tile([1, 64], F32)
            nc.vector.memset(ones, 1.0)

            w_sb = {}
            for name, dram in (("wq", wqt), ("wk", wkt), ("wv", wvt)):
                w = singles.tile([128, DC, CW], BF16, name=name + "_sb")
                nc.sync.dma_start(w, dram.rearrange("(c p) h -> p c h", p=128))
                w_sb[name] = w
            wot_sb = singles.tile([128, 2, D], BF16)
            nc.sync.dma_start(wot_sb, wot.rearrange("(g p) d -> p g d", p=128))
            b_sb = {}
            for name, dram in (("bq", bq), ("bk", bk)):
                b = singles.tile([128, 2], F32, name=name + "_sb")
                nc.sync.dma_start(b, dram.rearrange("(g p) one -> p (g one)", p=128))
                b_sb[name] = b
            bvb_sb = singles.tile([128, CW], F32)
            nc.sync.dma_start(bvb_sb, bvb)

            qm = acts.tile([128, 2, L], BF16)  # feature-major projections
            km = acts.tile([128, 2, L], BF16)
            vma = acts.tile([128, NT, HG * 66], BF16)  # [key, kt, (hv|1) x4 heads]
            ctxn = acts.tile([128, 2, L], BF16)

            # ones columns of the augmented V (col 64 of each head's 66)
            nc.vector.memset(
                vma.rearrange("p t (h c) -> p t h c", c=66)[:, :, :, 64], 1.0
            )

            # ---- phase 1: K/V then Q projections, per 512-token chunk ----
            for name, dram in (("k", kT), ("v", vT), ("q", qT)):
                for ci in range(NCH):
                    ts = slice(ci * 512, (ci + 1) * 512)
                    xc = stage.tile([128, DC, 512], BF16, tag="xc", name=f"{name}c")
                    nc.sync.dma_start(
                        xc, dram.rearrange("(c p) t -> p c t", p=128)[:, :, ts]
                    )
                    if name == "v":
                        # token-major: vm[t, f], activations as the
                        # stationary operand
                        for tt in range(4):
                            ps = ppq.tile([128, 2, 512], F32, tag="sq")
                            for dc in range(DC):
                                nc.tensor.matmul(
                                    ps[:, 0, 0:CW],
                                    lhsT=xc[:, dc, tt * 128 : (tt + 1) * 128],
                                    rhs=w_sb["wv"][:, dc, :],
                                    start=(dc == 0),
                                    stop=(dc == DC - 1),
                                )
                            nc.vector.tensor_add(
                                vma.rearrange("p t (h c) -> p t h c", c=66)[
                                    :, ci * 4 + tt, :, 0:64
                                ],
                                ps[:, 0, :].rearrange("p (h c) -> p h c", c=64)[
                                    :, 0:HG, :
                                ],
                                bvb_sb.rearrange("p (h c) -> p h c", c=64),
                            )
                    else:
                        dst = km if name == "k" else qm
                        bias = b_sb["bk" if name == "k" else "bq"]
                        for g in range(2):
                            ps = ppq.tile([128, 2, 512], F32, tag="sq")
                            for dc in range(DC):
                                nc.tensor.matmul(
                                    ps[:, 0, :],
                                    lhsT=w_sb["w" + name][
                                        :, dc, g * 128 : (g + 1) * 128
                                    ],
                                    rhs=xc[:, dc, :],
                                    start=(dc == 0),
                                    stop=(dc == DC - 1),
                                )
                            nc.vector.tensor_scalar_add(
                                dst[:, g, ts], ps[:, 0, :], bias[:, g : g + 1]
                            )

            # ---- phase 2: attention per (head pair g, 512-query chunk) ----
            for g in range(2):
                for lc in range(NCH):
                    ls = slice(lc * 512, (lc + 1) * 512)
                    ctx = [
                        ppc.tile([128, 512], F32, tag="ctx", name=f"ctx{h}")
                        for h in range(2)
                    ]
                    for pt in range(NT):
                        ks = slice(pt * 128, (pt + 1) * 128)
                        squad = ppq.tile([128, 2, 512], F32, tag="sq")
                        for h in range(2):
                            hs = slice(h * 64, (h + 1) * 64)
                            nc.tensor.matmul(
                                squad[:, h, :],
                                lhsT=km[hs, g, ks],
                                rhs=qm[hs, g, ls],
                                tile_position=(h * 64, 0),
                            )
                        es = small.tile([128, 2, 512], BF16, tag="es")
                        nc.scalar.activation(
                            es, squad, mybir.ActivationFunctionType.Exp, scale=SCALE
                        )
                        for h in range(2):
                            nc.tensor.matmul(
                                ctx[h][0:65, :],
                                lhsT=vma[
                                    :, pt, (g * 2 + h) * 66 : (g * 2 + h) * 66 + 65
                                ],
                                rhs=es[:, h, :],
                                start=(pt == 0),
                                stop=(pt == NT - 1),
                            )
                    for h in range(2):
                        rc = small.tile([1, 512], F32, tag="rc")
                        nc.vector.reciprocal(rc, ctx[h][64:65, :])
                        nc.tensor.matmul(ctx[h][64:128, :], lhsT=ones, rhs=rc)
                        bcs = small.tile([64, 512], F32, tag="bcs")
                        nc.vector.tensor_copy(bcs, ctx[h][64:128, :])
                        nc.vector.tensor_mul(
                            ctxn[h * 64 : (h + 1) * 64, g, ls], ctx[h][0:64, :], bcs
                        )

            # ---- phase 3: out-projection, per 128-token tile ----
            for lc in range(NCH):
                for tt in range(4):
                    t0 = lc * 512 + tt * 128
                    ob = small.tile([128, D], F32, tag="ob")
                    for half in range(2):
                        po = ppq.tile([128, 2, 512], F32, tag="sq")
                        for g in range(2):
                            nc.tensor.matmul(
                                po[:, 0, :],
                                lhsT=ctxn[:, g, t0 : t0 + 128],
                                rhs=wot_sb[:, g, half * 512 : (half + 1) * 512],
                                start=(g == 0),
                                stop=(g == 1),
                            )
                        nc.vector.tensor_copy(
                            ob[:, half * 512 : (half + 1) * 512], po[:, 0, :]
                        )
                    nc.sync.dma_start(out_p[t0 : t0 + 128, :], ob)
    return _split_matmul_waits(nc)


_NC_CACHE = None


def kernel(q, k, v, attention_mask, Wq, bq, Wk, bk, Wv, bv, Wo, bo):
    global _NC_CACHE
    q, k, v = (np.asarray(x, np.float32) for x in (q, k, v))
    assert np.asarray(attention_mask).all(), "kernel assumes all-ones mask"
    if _NC_CACHE is None:
        _NC_CACHE = build_nc()
    nc = _NC_CACHE

    bfc = lambda x: np.ascontiguousarray(np.asarray(x, ml_dtypes.bfloat16))
    c = np.ascontiguousarray
    Wq, Wk, Wv, Wo = (np.asarray(x, np.float32) for x in (Wq, Wk, Wv, Wo))
    bq, bk, bv, bo = (np.asarray(x, np.float32) for x in (bq, bk, bv, bo))

    qT = [bfc(q[b].T) for b in range(B)]
    kT = [bfc(k[b].T) for b in range(B)]
    vT = [bfc(v[b].T) for b in range(B)]

    in_maps = []
    for ci in range(N_CORES):
        b = ci // (N_CORES // B)
        j = ci % (N_CORES // B)
        hs = slice(j * CW, (j + 1) * CW)
        in_maps.append(
            {
                "qT": qT[b],
                "kT": kT[b],
                "vT": vT[b],
                "wqt": bfc(Wq.T[:, hs]),
                "wkt": bfc(Wk.T[:, hs]),
                "wvt": bfc(Wv.T[:, hs]),
                "wot": bfc(Wo.T[hs, :]),
                "bq": c(bq[hs, None]),
                "bk": c(bk[hs, None]),
                "bvb": c(np.broadcast_to(bv[hs][None, :], (128, CW))),
            }
        )

    res = bass_utils.run_bass_kernel_spmd(
        nc,
        in_maps,
        core_ids=list(range(N_CORES)),
        tmpdir=os.environ.get("KERNEL_TMPDIR"),
    )
    globals()["LAST_RES"] = res
    out = np.zeros((B, L, D), np.float32)
    for ci, r in enumerate(res.results):
        out[ci // (N_CORES // B)] += r["out_p"]
    out += bo[None, None, :]
    return out
